# revision 25
# baseline (speedup 1.0000x reference)
"""Trainium2 Bass kernel for EnhanceLayerLinear.

Computes out = GroupedLinear(Linear(x)):
    y = x @ W.T + b                      [B,S,D]
    out[..., g, :] = y[..., g, :] @ Wg[g].T + bg[g]   (block-diagonal, G groups)

Sharding: data-parallel over tokens (B*S = 8192 -> 1024 per core). Each core
runs both GEMM stages locally; the grouped stage shards trivially since it is
applied per token.

Stage 1 is a hybrid-precision GEMM: 20 of 32 contraction k-tiles run in bf16
(fp32 accumulate in psum) and the tail 12 k-tiles (6 pairs) run as fp8-e4m3
DoubleRow matmuls, which process two 128-row k-tiles per 512-column pass --
1.8x the bf16 MAC rate in situ (235 vs 216 ns issue-to-issue per matmul; the
DR weight load is FWL-disabled and costs ~22 ns/mm that neither weight reuse
nor SwInterleave removes). The fp8 fraction is capped at 37.5% of K by the
correctness gate: e4m3 carries ~2% relative error per operand, and the exact
(seed-fixed, deterministic on HW to ~1e-4 relative) end-to-end error of this
split is 1.682e-2 absmax-rel / 1.95e-2 L2 vs the 2e-2 limit; one more fp8
pair is 1.97e-2/2.1e-2 and over on L2. Both precisions share one psum
accumulation group because W is globally pre-scaled by 64 (exact in bf16;
lifts the fp8 copy of W, std 0.02, out of e4m3's subnormal range -- min
normal 2^-6) and the 1/64 descale rides the existing ACT evacuation for
free. Net: ~395 us vs 488.5 us for the all-bf16 ancestor, ~18 us of which
is fixed NEFF init/teardown plus ramp fill.

Stage 2 (the small grouped matmul) runs in bf16 off the psum evacuation: y is
quantized to bf16 (adds ~0.1% relative error, negligible vs the budget) so
each grouped matmul is a 213 ns PE slot with a hideable 1-pass LDWEIGHTS,
instead of f32r's 422 ns slot with an unhideable 2-pass weight load.

Layout trick: stage 1 computes y TRANSPOSED (features on partitions, tokens on
the free axis). That makes each 128-row psum tile exactly one group's slice
with the contraction axis of stage 2 already on partitions, so the grouped
matmul chains directly with zero on-chip transposes. The host hands the kernel
pre-transposed views of x / W / Wg and re-transposes the output.
"""

from collections import deque

import ml_dtypes
import numpy as np

import concourse.bacc as bacc
import concourse.bass as bass
import concourse.tile as tile
from concourse import mybir
from concourse import bass_utils

f32 = mybir.dt.float32
bf16 = mybir.dt.bfloat16
f8e4 = mybir.dt.float8e4
ACT_ID = mybir.ActivationFunctionType.Identity
DR = mybir.MatmulPerfMode.DoubleRow

B, S, D = 4, 2048, 4096
T = B * S                 # 8192 tokens
G, IG = 32, 128           # groups x group size (4096 = 32*128)
NCORES = 8
TPC = T // NCORES         # 1024 tokens per core
KT = D // 128             # 32 contraction tiles
NPAIRS = 6                # fp8 DoubleRow k-tile pairs (tail of K)
KT_BF = KT - 2 * NPAIRS   # 20 leading bf16 k-tiles
K_BF = KT_BF * 128        # 2560
WSCALE = 64.0             # global W pre-scale (power of 2, exact in bf16)
NMOV = 512                # moving free dim per matmul (= one psum bank of fp32)
NCH = TPC // NMOV         # 2 token chunks per core

_CACHE = {}


def _build():
    nc = bacc.Bacc("TRN2", target_bir_lowering=False, debug=False)
    # xb_d[kt, tch, p, t] = bf16 x[core_t0 + tch*512 + t, kt*128 + p]
    # x8_d[tch, p, pr, i, t] = e4m3 x[.. + t, K_BF + (pr*2+i)*128 + p]
    # wb_d[og, p, kt*128 + o] = bf16 64*W[og*128 + o, kt*128 + p]
    # w8_d[og, p, pr, i, o] = e4m3 64*W[og*128 + o, K_BF + (pr*2+i)*128 + p]
    # wg_d[i, g*128 + o] = bf16 Wg[g, o, i]
    # b_d[i, g] = b[g*128 + i];  bg_d[o, g] = bg[g, o]
    xb_d = nc.dram_tensor(
        "xb", [KT_BF // 4, 128, 4, NCH * NMOV], bf16, kind="ExternalInput"
    )
    x8_d = nc.dram_tensor("x8", [128, NPAIRS, 2, NCH * NMOV], f8e4, kind="ExternalInput")
    wb_d = nc.dram_tensor("wb", [G, 128, K_BF], bf16, kind="ExternalInput")
    w8_d = nc.dram_tensor("w8", [G, 128, NPAIRS, 2, 128], f8e4, kind="ExternalInput")
    wg_d = nc.dram_tensor("wg", [128, G * IG], bf16, kind="ExternalInput")
    b_d = nc.dram_tensor("b", [128, G], f32, kind="ExternalInput")
    bg_d = nc.dram_tensor("bg", [128, G], f32, kind="ExternalInput")
    # o_d[og, o, t] = out[core_t0 + t, og*128 + o]                (outT)
    o_d = nc.dram_tensor("o", [G, 128, TPC], f32, kind="ExternalOutput")

    with tile.TileContext(nc) as tc:
        with (
            tc.tile_pool(name="xp", bufs=KT_BF // 4) as xp,
            tc.tile_pool(name="x8p", bufs=NCH) as x8p,
            tc.tile_pool(name="wp", bufs=6) as wp,
            tc.tile_pool(name="w8p", bufs=6) as w8p,
            tc.tile_pool(name="cp", bufs=1) as cp,
            tc.tile_pool(name="yp", bufs=13) as yp,
            tc.tile_pool(name="op", bufs=5) as op,
            tc.tile_pool(name="ps1", bufs=4, space=bass.MemorySpace.PSUM) as ps1,
            tc.tile_pool(name="ps2", bufs=4, space=bass.MemorySpace.PSUM) as ps2,
        ):
            w_tiles = {}

            def load_w(og):
                # W prefetch rides the Scalar engine's DMA queue: it has
                # passes of slack, and splitting issue across the two HWDGE
                # queues keeps the Sync queue free for x/out traffic.
                t = wp.tile([128, K_BF], bf16, tag="w")
                nc.scalar.dma_start(t[:], wb_d[og])
                t8 = w8p.tile([128, NPAIRS, 2, 128], f8e4, tag="w8")
                nc.scalar.dma_start(t8[:], w8_d[og])
                w_tiles[og] = (t, t8)

            # og-outer / token-chunk-inner: every weight slice feeds two
            # back-to-back matmuls (tch 0 then 1), so W streams from HBM once
            # and each LDWEIGHTS has a 2-matmul window to hide under -- this
            # is what keeps the fp8 DoubleRow weight loads (FWL-disabled,
            # ~140 ns) off the critical path.
            #
            # The ramp is DMA-bandwidth-bound, so queue order here IS the
            # schedule: x tiles and just-in-time W column chunks interleave
            # kt-by-kt so the PE can start after ~400 KB has landed, and the
            # first RAMP og-groups advance kt-major across 2*RAMP psum banks,
            # paced by the x-tile arrivals. The fp8 pair matmuls sit at the
            # tail of every group, so their (small) x/W tiles ride behind the
            # bf16 wave.
            RAMP = 4
            WCHUNK = 10           # kt-slices per ramp W chunk DMA
            b_sb = cp.tile([128, G], f32)
            ramp_w = []
            for og in range(RAMP):
                t = wp.tile([128, K_BF], bf16, tag="w")
                ramp_w.append(t)
            x_sb = [None] * KT_BF
            wg_sb = cp.tile([128, G * IG], bf16)
            bg_sb = cp.tile([128, G], f32)
            # x rides the Sync queue in 4-kt chunk tiles (one ~1 MB DMA
            # each -- per-DMA issue occupancy is ~620 ns, so fewer, larger
            # transfers fill faster), ramp W chunks ride the Scalar queue:
            # the two issue streams run in parallel.
            for c in range(KT_BF // 4):
                t = xp.tile([128, 4, NCH * NMOV], bf16, tag="x")
                nc.sync.dma_start(t[:], xb_d[c])
                for kt in range(c * 4, (c + 1) * 4):
                    x_sb[kt] = t[:, kt - c * 4]
            for c in range(KT_BF // WCHUNK):
                lo, hi = c * WCHUNK * 128, (c + 1) * WCHUNK * 128
                for og in range(RAMP):
                    nc.scalar.dma_start(
                        ramp_w[og][:, lo:hi], wb_d[og][:, lo:hi]
                    )
            ramp_w8 = []
            x8_sb = x8p.tile([128, NPAIRS, 2, NCH * NMOV], f8e4, tag="x8")
            nc.sync.dma_start(x8_sb[:], x8_d[:])
            nc.sync.dma_start(b_sb[:], b_d[:])
            nc.sync.dma_start(bg_sb[:], bg_d[:])
            for og in range(RAMP):
                t8 = w8p.tile([128, NPAIRS, 2, 128], f8e4, tag="w8")
                nc.scalar.dma_start(t8[:], w8_d[og])
                ramp_w8.append(t8)
                w_tiles[og] = (ramp_w[og], t8)
            # og4 must land before the ramp's PE work drains (~44us); og5/og6
            # and wg are not needed until later, so they queue behind it.
            load_w(RAMP)
            load_w(RAMP + 1)
            nc.scalar.dma_start(wg_sb[:], wg_d[:])
            load_w(RAMP + 2)

            pending_q = deque()
            FLUSH_LAG = 8

            def flush_stage2(p):
                y_sb, og2, tch2 = p
                acc2 = ps2.tile([128, NMOV], f32, tag="acc2")
                nc.tensor.matmul(
                    acc2[:],
                    wg_sb[:, og2 * IG:(og2 + 1) * IG],
                    y_sb[:],
                    start=True,
                    stop=True,
                )
                o_sb = op.tile([128, NMOV], f32, tag="o")
                # evac on the (otherwise idle) DVE so the Scalar FIFO stays
                # free for the y activations the stage-2 matmuls wait on
                nc.vector.tensor_scalar_add(
                    o_sb[:], acc2[:], bg_sb[:, og2:og2 + 1]
                )
                nc.sync.dma_start(
                    o_d[og2][:, tch2 * NMOV:(tch2 + 1) * NMOV], o_sb[:]
                )

            def emit_y(acc, og, tch):
                y_sb = yp.tile([128, NMOV], bf16, tag="y")
                nc.scalar.activation(
                    y_sb[:], acc[:], ACT_ID,
                    bias=b_sb[:, og:og + 1], scale=1.0 / WSCALE,
                )
                pending_q.append((y_sb, og, tch))

            def mm_group_pair(accs2, wpair):
                # one og, both token chunks. The bf16 run goes sequentially
                # per psum bank (per-matmul bank alternation costs ~4 ns/mm
                # in HAM micro-idles); the fp8 DoubleRow tail alternates so
                # consecutive matmuls share weights, which keeps the
                # FWL-disabled DR weight loads hidden.
                w_sb, w8_sb = wpair
                for tch in range(NCH):
                    for kt in range(KT_BF):
                        nc.tensor.matmul(
                            accs2[tch][:],
                            w_sb[:, kt * 128:(kt + 1) * 128],
                            x_sb[kt][:, tch * NMOV:(tch + 1) * NMOV],
                            start=(kt == 0),
                            stop=False,
                        )
                for pr in range(NPAIRS):
                    for tch in range(NCH):
                        nc.tensor.matmul(
                            accs2[tch][:],
                            w8_sb[:, pr],
                            x8_sb[:, pr, :, tch * NMOV:(tch + 1) * NMOV],
                            start=False,
                            stop=(pr == NPAIRS - 1),
                            perf_mode=DR,
                        )

            # Interleaved ramp: RAMP og-groups x 2 chunks advance together,
            # kt-major, one psum bank each, paced by the x-tile arrivals.
            accs = [
                [ps1.tile([128, NMOV], f32, tag="acc", name=f"racc{_r}"),
                 ps2.tile([128, NMOV], f32, tag="acc2", name=f"racc2_{_r}")]
                for _r in range(RAMP)
            ]
            for kt in range(KT_BF):
                for og in range(RAMP):
                    for tch in range(NCH):
                        nc.tensor.matmul(
                            accs[og][tch][:],
                            ramp_w[og][:, kt * 128:(kt + 1) * 128],
                            x_sb[kt][:, tch * NMOV:(tch + 1) * NMOV],
                            start=(kt == 0),
                            stop=False,
                        )
            for pr in range(NPAIRS):
                for og in range(RAMP):
                    for tch in range(NCH):
                        nc.tensor.matmul(
                            accs[og][tch][:],
                            ramp_w8[og][:, pr],
                            x8_sb[:, pr, :, tch * NMOV:(tch + 1) * NMOV],
                            start=False,
                            stop=(pr == NPAIRS - 1),
                            perf_mode=DR,
                        )
            for og in range(RAMP):
                for tch in range(NCH):
                    emit_y(accs[og][tch], og, tch)

            for og in range(RAMP, G):
                wpair = w_tiles.pop(og)
                if og + 3 < G:
                    load_w(og + 3)
                accs2 = [
                    ps1.tile([128, NMOV], f32, tag="acc", name=f"acc{og}_0"),
                    ps1.tile([128, NMOV], f32, tag="acc", name=f"acc{og}_1"),
                ]
                # Earlier groups' stage-2 matmuls go out first (their y is
                # ready passes ago): at pass start they slide into the PE
                # queue before this group's stage-1 stream instead of
                # clustering at the boundary with the emit_y/start matmuls.
                # The lag defers the first use of wg past the DMA-bound
                # ramp; near the end drain faster so almost nothing is left
                # after the last stage-1 matmul.
                budget = 2 if og < G - 3 else 4
                floor = FLUSH_LAG if og < G - 3 else 1
                while budget and len(pending_q) >= floor:
                    flush_stage2(pending_q.popleft())
                    budget -= 1
                mm_group_pair(accs2, wpair)
                emit_y(accs2[0], og, 0)
                emit_y(accs2[1], og, 1)
            while pending_q:
                flush_stage2(pending_q.popleft())

    nc.compile()
    return nc


def _get_nc():
    if "nc" not in _CACHE:
        _CACHE["nc"] = _build()
    return _CACHE["nc"]


def _run(x, W, b, Wg, bg, trace=False, tmpdir=None):
    x = np.ascontiguousarray(x, dtype=np.float32)
    W = np.ascontiguousarray(W, dtype=np.float32)
    b = np.ascontiguousarray(b, dtype=np.float32)
    Wg = np.ascontiguousarray(Wg, dtype=np.float32)
    bg = np.ascontiguousarray(bg, dtype=np.float32)

    def e4(a):
        return np.clip(a, -240.0, 240.0).astype(ml_dtypes.float8_e4m3fn)

    # Host-side layout prep (permutes + dtype casts, no math).
    # x: [B,S,D] -> per-core xT tiles, bf16 head / e4m3 tail of K
    xt = x.reshape(NCORES, TPC, D)                         # [c, tok, k]
    xb_dev = np.ascontiguousarray(
        xt[..., :K_BF].reshape(NCORES, TPC, KT_BF, 128)
        .transpose(0, 2, 3, 1)                             # [c, kt, p, tok]
        .reshape(NCORES, KT_BF // 4, 4, 128, NCH * NMOV)
        .transpose(0, 1, 3, 2, 4)                          # [c, chunk, p, ktin, tok]
        .astype(ml_dtypes.bfloat16)
    )
    x8_dev = np.ascontiguousarray(
        e4(xt[..., K_BF:]).reshape(NCORES, TPC, NPAIRS, 2, 128)
        .transpose(0, 4, 2, 3, 1)                          # [c, p, pr, i, tok]
    )
    # W: [D_out, D_in] -> per-og kT-major slabs, pre-scaled by 64
    Ws = W * WSCALE
    wb_dev = np.ascontiguousarray(
        Ws[:, :K_BF].reshape(G, 128, KT_BF, 128).transpose(0, 3, 2, 1)
        .reshape(G, 128, K_BF).astype(ml_dtypes.bfloat16)
    )
    w8_dev = np.ascontiguousarray(
        e4(Ws[:, K_BF:]).reshape(G, 128, NPAIRS, 2, 128)
        .transpose(0, 4, 2, 3, 1)                          # [og, p, pr, i, o]
    )
    wg_dev = np.ascontiguousarray(
        Wg.transpose(2, 0, 1).reshape(128, G * IG).astype(ml_dtypes.bfloat16)
    )
    b_dev = np.ascontiguousarray(b.reshape(G, 128).T)
    bg_dev = np.ascontiguousarray(bg.T)

    in_maps = [
        {
            "xb": xb_dev[c], "x8": x8_dev[c], "wb": wb_dev, "w8": w8_dev,
            "wg": wg_dev, "b": b_dev, "bg": bg_dev,
        }
        for c in range(NCORES)
    ]
    nc = _get_nc()
    res = bass_utils.run_bass_kernel_spmd(
        nc, in_maps, core_ids=list(range(NCORES)), trace=trace, tmpdir=tmpdir
    )
    _CACHE["last_result"] = res

    out_t = np.concatenate(
        [res.results[c]["o"].reshape(D, TPC) for c in range(NCORES)], axis=1
    )
    return np.ascontiguousarray(out_t.T).reshape(B, S, D)


def kernel(x, W, b, Wg, bg):
    return _run(x, W, b, Wg, bg, trace=False)


# revision 28
# speedup vs baseline: 1.0202x; 1.0202x over previous
"""Trainium2 Bass kernel for EnhanceLayerLinear.

Computes out = GroupedLinear(Linear(x)):
    y = x @ W.T + b                      [B,S,D]
    out[..., g, :] = y[..., g, :] @ Wg[g].T + bg[g]   (block-diagonal, G groups)

Sharding: data-parallel over tokens (B*S = 8192 -> 1024 per core). Each core
runs both GEMM stages locally; the grouped stage shards trivially since it is
applied per token.

Stage 1 is a hybrid-precision GEMM: 20 of 32 contraction k-tiles run in bf16
(fp32 accumulate in psum) and the tail 12 k-tiles (6 pairs) run as fp8-e4m3
DoubleRow matmuls, which process two 128-row k-tiles per 512-column pass --
1.8x the bf16 MAC rate in situ (235 vs 216 ns issue-to-issue per matmul; the
DR weight load is FWL-disabled and costs ~22 ns/mm that neither weight reuse
nor SwInterleave removes). The fp8 fraction is capped at 37.5% of K by the
correctness gate: e4m3 carries ~2% relative error per operand, and the exact
(seed-fixed, deterministic on HW to ~1e-4 relative) end-to-end error of this
split is 1.682e-2 absmax-rel / 1.95e-2 L2 vs the 2e-2 limit; one more fp8
pair is 1.97e-2/2.1e-2 and over on L2. Both precisions share one psum
accumulation group because W is globally pre-scaled by 64 (exact in bf16;
lifts the fp8 copy of W, std 0.02, out of e4m3's subnormal range -- min
normal 2^-6) and the 1/64 descale rides the existing ACT evacuation for
free. Net: ~395 us vs 488.5 us for the all-bf16 ancestor, ~18 us of which
is fixed NEFF init/teardown plus ramp fill.

Stage 2 (the small grouped matmul) runs in bf16 off the psum evacuation: y is
quantized to bf16 (adds ~0.1% relative error, negligible vs the budget) so
each grouped matmul is a 213 ns PE slot with a hideable 1-pass LDWEIGHTS,
instead of f32r's 422 ns slot with an unhideable 2-pass weight load.

Layout trick: stage 1 computes y TRANSPOSED (features on partitions, tokens on
the free axis). That makes each 128-row psum tile exactly one group's slice
with the contraction axis of stage 2 already on partitions, so the grouped
matmul chains directly with zero on-chip transposes. The host hands the kernel
pre-transposed views of x / W / Wg and re-transposes the output.
"""

from collections import deque

import ml_dtypes
import numpy as np

import concourse.bacc as bacc
import concourse.bass as bass
import concourse.tile as tile
from concourse import mybir
from concourse import bass_utils

f32 = mybir.dt.float32
bf16 = mybir.dt.bfloat16
f8e4 = mybir.dt.float8e4
ACT_ID = mybir.ActivationFunctionType.Identity
DR = mybir.MatmulPerfMode.DoubleRow

B, S, D = 4, 2048, 4096
T = B * S                 # 8192 tokens
G, IG = 32, 128           # groups x group size (4096 = 32*128)
NCORES = 8
TPC = T // NCORES         # 1024 tokens per core
KT = D // 128             # 32 contraction tiles
NPAIRS = 6                # fp8 DoubleRow k-tile pairs (tail of K)
KT_BF = KT - 2 * NPAIRS   # 20 leading bf16 k-tiles
K_BF = KT_BF * 128        # 2560
WSCALE = 64.0             # global W pre-scale (power of 2, exact in bf16)
NMOV = 512                # moving free dim per matmul (= one psum bank of fp32)
NCH = TPC // NMOV         # 2 token chunks per core

_CACHE = {}


def _build():
    nc = bacc.Bacc("TRN2", target_bir_lowering=False, debug=False)
    # xb_d[kt, tch, p, t] = bf16 x[core_t0 + tch*512 + t, kt*128 + p]
    # x8_d[tch, p, pr, i, t] = e4m3 x[.. + t, K_BF + (pr*2+i)*128 + p]
    # wb_d[og, p, kt*128 + o] = bf16 64*W[og*128 + o, kt*128 + p]
    # w8_d[og, p, pr, i, o] = e4m3 64*W[og*128 + o, K_BF + (pr*2+i)*128 + p]
    # wg_d[i, g*128 + o] = bf16 Wg[g, o, i]
    # b_d[i, g] = b[g*128 + i];  bg_d[o, g] = bg[g, o]
    xb_d = nc.dram_tensor(
        "xb", [KT_BF // 4, 128, 4, NCH * NMOV], bf16, kind="ExternalInput"
    )
    x8_d = nc.dram_tensor("x8", [128, NPAIRS, 2, NCH * NMOV], f8e4, kind="ExternalInput")
    wb_d = nc.dram_tensor("wb", [G, 128, K_BF], bf16, kind="ExternalInput")
    w8_d = nc.dram_tensor("w8", [G, 128, NPAIRS, 2, 128], f8e4, kind="ExternalInput")
    wg_d = nc.dram_tensor("wg", [128, G * IG], bf16, kind="ExternalInput")
    b_d = nc.dram_tensor("b", [128, G], f32, kind="ExternalInput")
    bg_d = nc.dram_tensor("bg", [128, G], f32, kind="ExternalInput")
    # o_d[og, o, t] = out[core_t0 + t, og*128 + o]                (outT)
    o_d = nc.dram_tensor("o", [G, 128, TPC], f32, kind="ExternalOutput")

    with tile.TileContext(nc) as tc:
        with (
            tc.tile_pool(name="xp", bufs=KT_BF // 4) as xp,
            tc.tile_pool(name="x8p", bufs=NCH) as x8p,
            tc.tile_pool(name="wp", bufs=6) as wp,
            tc.tile_pool(name="w8p", bufs=6) as w8p,
            tc.tile_pool(name="cp", bufs=1) as cp,
            tc.tile_pool(name="yp", bufs=13) as yp,
            tc.tile_pool(name="op", bufs=5) as op,
            tc.tile_pool(name="ps1", bufs=4, space=bass.MemorySpace.PSUM) as ps1,
            tc.tile_pool(name="ps2", bufs=4, space=bass.MemorySpace.PSUM) as ps2,
        ):
            w_tiles = {}

            def load_w(og):
                # W prefetch rides the Scalar engine's DMA queue: it has
                # passes of slack, and splitting issue across the two HWDGE
                # queues keeps the Sync queue free for x/out traffic.
                t = wp.tile([128, K_BF], bf16, tag="w")
                nc.scalar.dma_start(t[:], wb_d[og])
                t8 = w8p.tile([128, NPAIRS, 2, 128], f8e4, tag="w8")
                nc.scalar.dma_start(t8[:], w8_d[og])
                w_tiles[og] = (t, t8)

            # og-outer / token-chunk-inner: every weight slice feeds two
            # back-to-back matmuls (tch 0 then 1), so W streams from HBM once
            # and each LDWEIGHTS has a 2-matmul window to hide under -- this
            # is what keeps the fp8 DoubleRow weight loads (FWL-disabled,
            # ~140 ns) off the critical path.
            #
            # The ramp is DMA-bandwidth-bound, so queue order here IS the
            # schedule: x tiles and just-in-time W column chunks interleave
            # kt-by-kt so the PE can start after ~400 KB has landed, and the
            # first RAMP og-groups advance kt-major across 2*RAMP psum banks,
            # paced by the x-tile arrivals. The fp8 pair matmuls sit at the
            # tail of every group, so their (small) x/W tiles ride behind the
            # bf16 wave.
            RAMP = 4
            WCHUNK = 4            # kt-slices per ramp W + x chunk DMA
            b_sb = cp.tile([128, G], f32)
            ramp_w = []
            for og in range(RAMP):
                t = wp.tile([128, K_BF], bf16, tag="w")
                ramp_w.append(t)
            x_sb = [None] * KT_BF
            wg_sb = cp.tile([128, G * IG], bf16)
            bg_sb = cp.tile([128, G], f32)
            # x rides the Sync queue in 4-kt chunk tiles (one ~1 MB DMA
            # each -- per-DMA issue occupancy is ~620 ns, so fewer, larger
            # transfers fill faster), ramp W chunks ride the Scalar queue.
            # The two streams MUST stay interleaved x-chunk-by-x-chunk:
            # transfers share DMA bandwidth across queues roughly in issue
            # order, so queueing all of x first starves the first W chunk
            # and idles the PE ~10us (measured).
            for c in range(KT_BF // WCHUNK):
                lo, hi = c * WCHUNK * 128, (c + 1) * WCHUNK * 128
                t = xp.tile([128, WCHUNK, NCH * NMOV], bf16, tag="x")
                nc.sync.dma_start(t[:], xb_d[c])
                for kt in range(c * WCHUNK, (c + 1) * WCHUNK):
                    x_sb[kt] = t[:, kt - c * WCHUNK]
                for og in range(RAMP):
                    nc.scalar.dma_start(
                        ramp_w[og][:, lo:hi], wb_d[og][:, lo:hi]
                    )
            # PE warm-up on scratch data during the initial DMA fill: the
            # HAM clock gate needs ~3-4us of sustained activity to release
            # full rate, which would otherwise be paid at the ramp's start.
            warm = cp.tile([128, NMOV], bf16, name="warm")
            nc.vector.memset(warm[:], 0.0)
            wacc = ps2.tile([128, NMOV], f32, tag="acc2", name="wacc")
            for wi in range(16):
                nc.tensor.matmul(
                    wacc[:], warm[:, 0:128], warm[:],
                    start=(wi == 0), stop=(wi == 15),
                )
            ramp_w8 = []
            x8_sb = x8p.tile([128, NPAIRS, 2, NCH * NMOV], f8e4, tag="x8")
            nc.sync.dma_start(x8_sb[:], x8_d[:])
            nc.sync.dma_start(b_sb[:], b_d[:])
            nc.sync.dma_start(bg_sb[:], bg_d[:])
            for og in range(RAMP):
                t8 = w8p.tile([128, NPAIRS, 2, 128], f8e4, tag="w8")
                nc.scalar.dma_start(t8[:], w8_d[og])
                ramp_w8.append(t8)
                w_tiles[og] = (ramp_w[og], t8)
            # og4 must land before the ramp's PE work drains (~44us); og5/og6
            # and wg are not needed until later, so they queue behind it.
            load_w(RAMP)
            load_w(RAMP + 1)
            nc.scalar.dma_start(wg_sb[:], wg_d[:])
            load_w(RAMP + 2)

            pending_q = deque()
            FLUSH_LAG = 8

            def flush_stage2(p):
                y_sb, og2, tch2 = p
                acc2 = ps2.tile([128, NMOV], f32, tag="acc2")
                nc.tensor.matmul(
                    acc2[:],
                    wg_sb[:, og2 * IG:(og2 + 1) * IG],
                    y_sb[:],
                    start=True,
                    stop=True,
                )
                o_sb = op.tile([128, NMOV], f32, tag="o")
                # evac on the (otherwise idle) DVE so the Scalar FIFO stays
                # free for the y activations the stage-2 matmuls wait on
                nc.vector.tensor_scalar_add(
                    o_sb[:], acc2[:], bg_sb[:, og2:og2 + 1]
                )
                nc.sync.dma_start(
                    o_d[og2][:, tch2 * NMOV:(tch2 + 1) * NMOV], o_sb[:]
                )

            def emit_y(acc, og, tch):
                y_sb = yp.tile([128, NMOV], bf16, tag="y")
                nc.scalar.activation(
                    y_sb[:], acc[:], ACT_ID,
                    bias=b_sb[:, og:og + 1], scale=1.0 / WSCALE,
                )
                pending_q.append((y_sb, og, tch))

            def mm_group_pair(accs2, wpair):
                # one og, both token chunks. The bf16 run goes sequentially
                # per psum bank (per-matmul bank alternation costs ~4 ns/mm
                # in HAM micro-idles); the fp8 DoubleRow tail alternates so
                # consecutive matmuls share weights, which keeps the
                # FWL-disabled DR weight loads hidden.
                w_sb, w8_sb = wpair
                for tch in range(NCH):
                    for kt in range(KT_BF):
                        nc.tensor.matmul(
                            accs2[tch][:],
                            w_sb[:, kt * 128:(kt + 1) * 128],
                            x_sb[kt][:, tch * NMOV:(tch + 1) * NMOV],
                            start=(kt == 0),
                            stop=False,
                        )
                for pr in range(NPAIRS):
                    for tch in range(NCH):
                        nc.tensor.matmul(
                            accs2[tch][:],
                            w8_sb[:, pr],
                            x8_sb[:, pr, :, tch * NMOV:(tch + 1) * NMOV],
                            start=False,
                            stop=(pr == NPAIRS - 1),
                            perf_mode=DR,
                        )

            # Interleaved ramp: RAMP og-groups x 2 chunks advance together,
            # kt-major, one psum bank each, paced by the x-tile arrivals.
            accs = [
                [ps1.tile([128, NMOV], f32, tag="acc", name=f"racc{_r}"),
                 ps2.tile([128, NMOV], f32, tag="acc2", name=f"racc2_{_r}")]
                for _r in range(RAMP)
            ]
            for kt in range(KT_BF):
                for og in range(RAMP):
                    for tch in range(NCH):
                        nc.tensor.matmul(
                            accs[og][tch][:],
                            ramp_w[og][:, kt * 128:(kt + 1) * 128],
                            x_sb[kt][:, tch * NMOV:(tch + 1) * NMOV],
                            start=(kt == 0),
                            stop=False,
                        )
            for pr in range(NPAIRS):
                for og in range(RAMP):
                    for tch in range(NCH):
                        nc.tensor.matmul(
                            accs[og][tch][:],
                            ramp_w8[og][:, pr],
                            x8_sb[:, pr, :, tch * NMOV:(tch + 1) * NMOV],
                            start=False,
                            stop=(pr == NPAIRS - 1),
                            perf_mode=DR,
                        )
            for og in range(RAMP):
                for tch in range(NCH):
                    emit_y(accs[og][tch], og, tch)

            for og in range(RAMP, G):
                wpair = w_tiles.pop(og)
                if og + 3 < G:
                    load_w(og + 3)
                accs2 = [
                    ps1.tile([128, NMOV], f32, tag="acc", name=f"acc{og}_0"),
                    ps1.tile([128, NMOV], f32, tag="acc", name=f"acc{og}_1"),
                ]
                # Earlier groups' stage-2 matmuls go out first (their y is
                # ready passes ago): at pass start they slide into the PE
                # queue before this group's stage-1 stream instead of
                # clustering at the boundary with the emit_y/start matmuls.
                # The lag defers the first use of wg past the DMA-bound
                # ramp; near the end drain faster so almost nothing is left
                # after the last stage-1 matmul.
                budget = 2 if og < G - 3 else 4
                floor = FLUSH_LAG if og < G - 3 else 1
                while budget and len(pending_q) >= floor:
                    flush_stage2(pending_q.popleft())
                    budget -= 1
                mm_group_pair(accs2, wpair)
                emit_y(accs2[0], og, 0)
                emit_y(accs2[1], og, 1)
            while pending_q:
                flush_stage2(pending_q.popleft())

    nc.compile()
    return nc


def _get_nc():
    if "nc" not in _CACHE:
        _CACHE["nc"] = _build()
    return _CACHE["nc"]


def _run(x, W, b, Wg, bg, trace=False, tmpdir=None):
    x = np.ascontiguousarray(x, dtype=np.float32)
    W = np.ascontiguousarray(W, dtype=np.float32)
    b = np.ascontiguousarray(b, dtype=np.float32)
    Wg = np.ascontiguousarray(Wg, dtype=np.float32)
    bg = np.ascontiguousarray(bg, dtype=np.float32)

    def e4(a):
        return np.clip(a, -240.0, 240.0).astype(ml_dtypes.float8_e4m3fn)

    # Host-side layout prep (permutes + dtype casts, no math).
    # x: [B,S,D] -> per-core xT tiles, bf16 head / e4m3 tail of K
    xt = x.reshape(NCORES, TPC, D)                         # [c, tok, k]
    xb_dev = np.ascontiguousarray(
        xt[..., :K_BF].reshape(NCORES, TPC, KT_BF, 128)
        .transpose(0, 2, 3, 1)                             # [c, kt, p, tok]
        .reshape(NCORES, KT_BF // 4, 4, 128, NCH * NMOV)
        .transpose(0, 1, 3, 2, 4)                          # [c, chunk, p, ktin, tok]
        .astype(ml_dtypes.bfloat16)
    )
    x8_dev = np.ascontiguousarray(
        e4(xt[..., K_BF:]).reshape(NCORES, TPC, NPAIRS, 2, 128)
        .transpose(0, 4, 2, 3, 1)                          # [c, p, pr, i, tok]
    )
    # W: [D_out, D_in] -> per-og kT-major slabs, pre-scaled by 64
    Ws = W * WSCALE
    wb_dev = np.ascontiguousarray(
        Ws[:, :K_BF].reshape(G, 128, KT_BF, 128).transpose(0, 3, 2, 1)
        .reshape(G, 128, K_BF).astype(ml_dtypes.bfloat16)
    )
    w8_dev = np.ascontiguousarray(
        e4(Ws[:, K_BF:]).reshape(G, 128, NPAIRS, 2, 128)
        .transpose(0, 4, 2, 3, 1)                          # [og, p, pr, i, o]
    )
    wg_dev = np.ascontiguousarray(
        Wg.transpose(2, 0, 1).reshape(128, G * IG).astype(ml_dtypes.bfloat16)
    )
    b_dev = np.ascontiguousarray(b.reshape(G, 128).T)
    bg_dev = np.ascontiguousarray(bg.T)

    in_maps = [
        {
            "xb": xb_dev[c], "x8": x8_dev[c], "wb": wb_dev, "w8": w8_dev,
            "wg": wg_dev, "b": b_dev, "bg": bg_dev,
        }
        for c in range(NCORES)
    ]
    nc = _get_nc()
    res = bass_utils.run_bass_kernel_spmd(
        nc, in_maps, core_ids=list(range(NCORES)), trace=trace, tmpdir=tmpdir
    )
    _CACHE["last_result"] = res

    out_t = np.concatenate(
        [res.results[c]["o"].reshape(D, TPC) for c in range(NCORES)], axis=1
    )
    return np.ascontiguousarray(out_t.T).reshape(B, S, D)


def kernel(x, W, b, Wg, bg):
    return _run(x, W, b, Wg, bg, trace=False)


# revision 32
# speedup vs baseline: 1.0211x; 1.0009x over previous
"""Trainium2 Bass kernel for EnhanceLayerLinear.

Computes out = GroupedLinear(Linear(x)):
    y = x @ W.T + b                      [B,S,D]
    out[..., g, :] = y[..., g, :] @ Wg[g].T + bg[g]   (block-diagonal, G groups)

Sharding: data-parallel over tokens (B*S = 8192 -> 1024 per core). Each core
runs both GEMM stages locally; the grouped stage shards trivially since it is
applied per token.

Stage 1 is a hybrid-precision GEMM: 20 of 32 contraction k-tiles run in bf16
(fp32 accumulate in psum) and the tail 12 k-tiles (6 pairs) run as fp8-e4m3
DoubleRow matmuls, which process two 128-row k-tiles per 512-column pass --
1.8x the bf16 MAC rate in situ (235 vs 216 ns issue-to-issue per matmul; the
DR weight load is FWL-disabled and costs ~22 ns/mm that neither weight reuse
nor SwInterleave removes). The fp8 fraction is capped at 37.5% of K by the
correctness gate: e4m3 carries ~2% relative error per operand, and the exact
(seed-fixed, deterministic on HW to ~1e-4 relative) end-to-end error of this
split is 1.682e-2 absmax-rel / 1.95e-2 L2 vs the 2e-2 limit; one more fp8
pair is 1.97e-2/2.1e-2 and over on L2. Both precisions share one psum
accumulation group because W is globally pre-scaled by 64 (exact in bf16;
lifts the fp8 copy of W, std 0.02, out of e4m3's subnormal range -- min
normal 2^-6) and the 1/64 descale rides the existing ACT evacuation for
free. Net: ~395 us vs 488.5 us for the all-bf16 ancestor, ~18 us of which
is fixed NEFF init/teardown plus ramp fill.

Stage 2 (the small grouped matmul) runs in bf16 off the psum evacuation: y is
quantized to bf16 (adds ~0.1% relative error, negligible vs the budget) so
each grouped matmul is a 213 ns PE slot with a hideable 1-pass LDWEIGHTS,
instead of f32r's 422 ns slot with an unhideable 2-pass weight load.

Layout trick: stage 1 computes y TRANSPOSED (features on partitions, tokens on
the free axis). That makes each 128-row psum tile exactly one group's slice
with the contraction axis of stage 2 already on partitions, so the grouped
matmul chains directly with zero on-chip transposes. The host hands the kernel
pre-transposed views of x / W / Wg and re-transposes the output.
"""

from collections import deque

import ml_dtypes
import numpy as np

import concourse.bacc as bacc
import concourse.bass as bass
import concourse.tile as tile
from concourse import mybir
from concourse import bass_utils

f32 = mybir.dt.float32
bf16 = mybir.dt.bfloat16
f8e4 = mybir.dt.float8e4
ACT_ID = mybir.ActivationFunctionType.Identity
DR = mybir.MatmulPerfMode.DoubleRow

B, S, D = 4, 2048, 4096
T = B * S                 # 8192 tokens
G, IG = 32, 128           # groups x group size (4096 = 32*128)
NCORES = 8
TPC = T // NCORES         # 1024 tokens per core
KT = D // 128             # 32 contraction tiles
NPAIRS = 6                # fp8 DoubleRow k-tile pairs (tail of K)
KT_BF = KT - 2 * NPAIRS   # 20 leading bf16 k-tiles
K_BF = KT_BF * 128        # 2560
WSCALE = 64.0             # global W pre-scale (power of 2, exact in bf16)
NMOV = 512                # moving free dim per matmul (= one psum bank of fp32)
NCH = TPC // NMOV         # 2 token chunks per core

_CACHE = {}


def _build():
    nc = bacc.Bacc("TRN2", target_bir_lowering=False, debug=False)
    # xb_d[kt, tch, p, t] = bf16 x[core_t0 + tch*512 + t, kt*128 + p]
    # x8_d[tch, p, pr, i, t] = e4m3 x[.. + t, K_BF + (pr*2+i)*128 + p]
    # wb_d[og, p, kt*128 + o] = bf16 64*W[og*128 + o, kt*128 + p]
    # w8_d[og, p, pr, i, o] = e4m3 64*W[og*128 + o, K_BF + (pr*2+i)*128 + p]
    # wg_d[i, g*128 + o] = bf16 Wg[g, o, i]
    # b_d[i, g] = b[g*128 + i];  bg_d[o, g] = bg[g, o]
    xb_d = nc.dram_tensor(
        "xb", [KT_BF // 4, 128, 4, NCH * NMOV], bf16, kind="ExternalInput"
    )
    x8_d = nc.dram_tensor("x8", [128, NPAIRS, 2, NCH * NMOV], f8e4, kind="ExternalInput")
    wb_d = nc.dram_tensor("wb", [G, 128, K_BF], bf16, kind="ExternalInput")
    w8_d = nc.dram_tensor("w8", [G, 128, NPAIRS, 2, 128], f8e4, kind="ExternalInput")
    wg_d = nc.dram_tensor("wg", [128, G * IG], bf16, kind="ExternalInput")
    b_d = nc.dram_tensor("b", [128, G], f32, kind="ExternalInput")
    bg_d = nc.dram_tensor("bg", [128, G], f32, kind="ExternalInput")
    # o_d[og, o, t] = out[core_t0 + t, og*128 + o]                (outT)
    o_d = nc.dram_tensor("o", [G, 128, TPC], f32, kind="ExternalOutput")

    with tile.TileContext(nc) as tc:
        with (
            tc.tile_pool(name="xp", bufs=KT_BF // 4) as xp,
            tc.tile_pool(name="x8p", bufs=1) as x8p,
            tc.tile_pool(name="wp", bufs=6) as wp,
            tc.tile_pool(name="w8p", bufs=6) as w8p,
            tc.tile_pool(name="cp", bufs=1) as cp,
            tc.tile_pool(name="yp", bufs=11) as yp,
            tc.tile_pool(name="op", bufs=4) as op,
            tc.tile_pool(name="ps1", bufs=4, space=bass.MemorySpace.PSUM) as ps1,
            tc.tile_pool(name="ps2", bufs=4, space=bass.MemorySpace.PSUM) as ps2,
        ):
            w_tiles = {}

            def load_w(og):
                # W prefetch rides the Scalar engine's DMA queue: it has
                # passes of slack, and splitting issue across the two HWDGE
                # queues keeps the Sync queue free for x/out traffic.
                t = wp.tile([128, K_BF], bf16, tag="w")
                nc.scalar.dma_start(t[:], wb_d[og])
                t8 = w8p.tile([128, NPAIRS, 2, 128], f8e4, tag="w8")
                nc.scalar.dma_start(t8[:], w8_d[og])
                w_tiles[og] = (t, t8)

            # og-outer / token-chunk-inner: every weight slice feeds two
            # back-to-back matmuls (tch 0 then 1), so W streams from HBM once
            # and each LDWEIGHTS has a 2-matmul window to hide under -- this
            # is what keeps the fp8 DoubleRow weight loads (FWL-disabled,
            # ~140 ns) off the critical path.
            #
            # The ramp is DMA-bandwidth-bound, so queue order here IS the
            # schedule: x tiles and just-in-time W column chunks interleave
            # kt-by-kt so the PE can start after ~400 KB has landed, and the
            # first RAMP og-groups advance kt-major across 2*RAMP psum banks,
            # paced by the x-tile arrivals. The fp8 pair matmuls sit at the
            # tail of every group, so their (small) x/W tiles ride behind the
            # bf16 wave.
            RAMP = 4
            WCHUNK = 4            # kt-slices per ramp W + x chunk DMA
            b_sb = cp.tile([128, G], f32)
            ramp_w = []
            for og in range(RAMP):
                t = wp.tile([128, K_BF], bf16, tag="w")
                ramp_w.append(t)
            x_sb = [None] * KT_BF
            wg_sb = cp.tile([128, G * IG], bf16)
            bg_sb = cp.tile([128, G], f32)
            # x rides the Sync queue in 4-kt chunk tiles (one ~1 MB DMA
            # each -- per-DMA issue occupancy is ~620 ns, so fewer, larger
            # transfers fill faster), ramp W chunks ride the Scalar queue.
            # The two streams MUST stay interleaved x-chunk-by-x-chunk:
            # transfers share DMA bandwidth across queues roughly in issue
            # order, so queueing all of x first starves the first W chunk
            # and idles the PE ~10us (measured).
            for c in range(KT_BF // WCHUNK):
                lo, hi = c * WCHUNK * 128, (c + 1) * WCHUNK * 128
                t = xp.tile([128, WCHUNK, NCH * NMOV], bf16, tag="x")
                nc.sync.dma_start(t[:], xb_d[c])
                for kt in range(c * WCHUNK, (c + 1) * WCHUNK):
                    x_sb[kt] = t[:, kt - c * WCHUNK]
                for og in range(RAMP):
                    nc.scalar.dma_start(
                        ramp_w[og][:, lo:hi], wb_d[og][:, lo:hi]
                    )
            # PE warm-up on scratch data during the initial DMA fill: the
            # HAM clock gate needs ~3-4us of sustained activity to release
            # full rate, which would otherwise be paid at the ramp's start.
            warm = cp.tile([128, NMOV], bf16, name="warm")
            nc.vector.memset(warm[:], 0.0)
            wacc = ps2.tile([128, NMOV], f32, tag="acc2", name="wacc")
            for wi in range(16):
                nc.tensor.matmul(
                    wacc[:], warm[:, 0:128], warm[:],
                    start=(wi == 0), stop=(wi == 15),
                )
            ramp_w8 = []
            x8_sb = x8p.tile([128, NPAIRS, 2, NCH * NMOV], f8e4, tag="x8")
            nc.sync.dma_start(x8_sb[:], x8_d[:])
            nc.sync.dma_start(b_sb[:], b_d[:])
            nc.sync.dma_start(bg_sb[:], bg_d[:])
            for og in range(RAMP):
                t8 = w8p.tile([128, NPAIRS, 2, 128], f8e4, tag="w8")
                nc.scalar.dma_start(t8[:], w8_d[og])
                ramp_w8.append(t8)
                w_tiles[og] = (ramp_w[og], t8)
            # og4 must land before the ramp's PE work drains (~44us); og5/og6
            # and wg are not needed until later, so they queue behind it.
            load_w(RAMP)
            load_w(RAMP + 1)
            nc.scalar.dma_start(wg_sb[:], wg_d[:])
            load_w(RAMP + 2)

            pending_q = deque()
            FLUSH_LAG = 8

            def flush_stage2_pair():
                # pop both token chunks of one og (always queued adjacently)
                # and emit them into one [128, TPC] tile with a single out
                # DMA -- halves the ~620 ns out-DMA issue slots.
                (y0, og2, _t0), (y1, og2b, _t1) = (
                    pending_q.popleft(), pending_q.popleft()
                )
                assert og2 == og2b
                o_sb = op.tile([128, TPC], f32, tag="o")
                for tch2, y_sb in ((0, y0), (1, y1)):
                    acc2 = ps2.tile([128, NMOV], f32, tag="acc2")
                    nc.tensor.matmul(
                        acc2[:],
                        wg_sb[:, og2 * IG:(og2 + 1) * IG],
                        y_sb[:],
                        start=True,
                        stop=True,
                    )
                    # evac on the (otherwise idle) DVE so the Scalar FIFO
                    # stays free for the y activations stage-2 waits on
                    nc.vector.tensor_scalar_add(
                        o_sb[:, tch2 * NMOV:(tch2 + 1) * NMOV], acc2[:],
                        bg_sb[:, og2:og2 + 1],
                    )
                nc.sync.dma_start(o_d[og2], o_sb[:])

            def emit_y(acc, og, tch):
                y_sb = yp.tile([128, NMOV], bf16, tag="y")
                nc.scalar.activation(
                    y_sb[:], acc[:], ACT_ID,
                    bias=b_sb[:, og:og + 1], scale=1.0 / WSCALE,
                )
                pending_q.append((y_sb, og, tch))

            def mm_group_pair(accs2, wpair):
                # one og, both token chunks. The bf16 run goes sequentially
                # per psum bank (per-matmul bank alternation costs ~4 ns/mm
                # in HAM micro-idles); the fp8 DoubleRow tail alternates so
                # consecutive matmuls share weights, which keeps the
                # FWL-disabled DR weight loads hidden.
                w_sb, w8_sb = wpair
                for tch in range(NCH):
                    for kt in range(KT_BF):
                        nc.tensor.matmul(
                            accs2[tch][:],
                            w_sb[:, kt * 128:(kt + 1) * 128],
                            x_sb[kt][:, tch * NMOV:(tch + 1) * NMOV],
                            start=(kt == 0),
                            stop=False,
                        )
                for pr in range(NPAIRS):
                    for tch in range(NCH):
                        nc.tensor.matmul(
                            accs2[tch][:],
                            w8_sb[:, pr],
                            x8_sb[:, pr, :, tch * NMOV:(tch + 1) * NMOV],
                            start=False,
                            stop=(pr == NPAIRS - 1),
                            perf_mode=DR,
                        )

            # Interleaved ramp: RAMP og-groups x 2 chunks advance together,
            # kt-major, one psum bank each, paced by the x-tile arrivals.
            accs = [
                [ps1.tile([128, NMOV], f32, tag="acc", name=f"racc{_r}"),
                 ps2.tile([128, NMOV], f32, tag="acc2", name=f"racc2_{_r}")]
                for _r in range(RAMP)
            ]
            for kt in range(KT_BF):
                for og in range(RAMP):
                    for tch in range(NCH):
                        nc.tensor.matmul(
                            accs[og][tch][:],
                            ramp_w[og][:, kt * 128:(kt + 1) * 128],
                            x_sb[kt][:, tch * NMOV:(tch + 1) * NMOV],
                            start=(kt == 0),
                            stop=False,
                        )
            for pr in range(NPAIRS):
                for og in range(RAMP):
                    for tch in range(NCH):
                        nc.tensor.matmul(
                            accs[og][tch][:],
                            ramp_w8[og][:, pr],
                            x8_sb[:, pr, :, tch * NMOV:(tch + 1) * NMOV],
                            start=False,
                            stop=(pr == NPAIRS - 1),
                            perf_mode=DR,
                        )
            for og in range(RAMP):
                for tch in range(NCH):
                    emit_y(accs[og][tch], og, tch)

            for og in range(RAMP, G):
                wpair = w_tiles.pop(og)
                if og + 3 < G:
                    load_w(og + 3)
                accs2 = [
                    ps1.tile([128, NMOV], f32, tag="acc", name=f"acc{og}_0"),
                    ps1.tile([128, NMOV], f32, tag="acc", name=f"acc{og}_1"),
                ]
                # Earlier groups' stage-2 matmuls go out first (their y is
                # ready passes ago): at pass start they slide into the PE
                # queue before this group's stage-1 stream instead of
                # clustering at the boundary with the emit_y/start matmuls.
                # The lag defers the first use of wg past the DMA-bound
                # ramp; near the end drain faster so almost nothing is left
                # after the last stage-1 matmul.
                budget = 1 if og < G - 3 else 2
                floor = FLUSH_LAG if og < G - 3 else 2
                while budget and len(pending_q) >= floor:
                    flush_stage2_pair()
                    budget -= 1
                mm_group_pair(accs2, wpair)
                emit_y(accs2[0], og, 0)
                emit_y(accs2[1], og, 1)
            while pending_q:
                flush_stage2_pair()

    nc.compile()
    return nc


def _get_nc():
    if "nc" not in _CACHE:
        _CACHE["nc"] = _build()
    return _CACHE["nc"]


def _run(x, W, b, Wg, bg, trace=False, tmpdir=None):
    x = np.ascontiguousarray(x, dtype=np.float32)
    W = np.ascontiguousarray(W, dtype=np.float32)
    b = np.ascontiguousarray(b, dtype=np.float32)
    Wg = np.ascontiguousarray(Wg, dtype=np.float32)
    bg = np.ascontiguousarray(bg, dtype=np.float32)

    def e4(a):
        return np.clip(a, -240.0, 240.0).astype(ml_dtypes.float8_e4m3fn)

    # Host-side layout prep (permutes + dtype casts, no math).
    # x: [B,S,D] -> per-core xT tiles, bf16 head / e4m3 tail of K
    xt = x.reshape(NCORES, TPC, D)                         # [c, tok, k]
    xb_dev = np.ascontiguousarray(
        xt[..., :K_BF].reshape(NCORES, TPC, KT_BF, 128)
        .transpose(0, 2, 3, 1)                             # [c, kt, p, tok]
        .reshape(NCORES, KT_BF // 4, 4, 128, NCH * NMOV)
        .transpose(0, 1, 3, 2, 4)                          # [c, chunk, p, ktin, tok]
        .astype(ml_dtypes.bfloat16)
    )
    x8_dev = np.ascontiguousarray(
        e4(xt[..., K_BF:]).reshape(NCORES, TPC, NPAIRS, 2, 128)
        .transpose(0, 4, 2, 3, 1)                          # [c, p, pr, i, tok]
    )
    # W: [D_out, D_in] -> per-og kT-major slabs, pre-scaled by 64
    Ws = W * WSCALE
    wb_dev = np.ascontiguousarray(
        Ws[:, :K_BF].reshape(G, 128, KT_BF, 128).transpose(0, 3, 2, 1)
        .reshape(G, 128, K_BF).astype(ml_dtypes.bfloat16)
    )
    w8_dev = np.ascontiguousarray(
        e4(Ws[:, K_BF:]).reshape(G, 128, NPAIRS, 2, 128)
        .transpose(0, 4, 2, 3, 1)                          # [og, p, pr, i, o]
    )
    wg_dev = np.ascontiguousarray(
        Wg.transpose(2, 0, 1).reshape(128, G * IG).astype(ml_dtypes.bfloat16)
    )
    b_dev = np.ascontiguousarray(b.reshape(G, 128).T)
    bg_dev = np.ascontiguousarray(bg.T)

    in_maps = [
        {
            "xb": xb_dev[c], "x8": x8_dev[c], "wb": wb_dev, "w8": w8_dev,
            "wg": wg_dev, "b": b_dev, "bg": bg_dev,
        }
        for c in range(NCORES)
    ]
    nc = _get_nc()
    res = bass_utils.run_bass_kernel_spmd(
        nc, in_maps, core_ids=list(range(NCORES)), trace=trace, tmpdir=tmpdir
    )
    _CACHE["last_result"] = res

    out_t = np.concatenate(
        [res.results[c]["o"].reshape(D, TPC) for c in range(NCORES)], axis=1
    )
    return np.ascontiguousarray(out_t.T).reshape(B, S, D)


def kernel(x, W, b, Wg, bg):
    return _run(x, W, b, Wg, bg, trace=False)
